# revision 16
# baseline (speedup 1.0000x reference)
"""3-layer GCN (100k nodes, 1.6M edges, 128->128->128->40) on 8 trn2 cores.

Self-contained harness kernel: kernel(**inputs) takes the FULL unsharded
inputs and returns the FULL [100000, 40] float32 output.

Strategy (1D node partition, edges sharded by dst):
  - nodes split contiguously across the 8 cores (12500 each, padded 12544);
    edges assigned to the core owning their dst.
  - per layer, each core computes the dense transform hp = (h @ W) * ns[row]
    on the PE (bf16 operands, f32 PSUM); layer 2 keeps a bf16 hi/lo split.
    Layer-0 ns is folded into featT host-side so the transform is a pure
    batched matmul + one Copy per 4 windows.
  - per-layer table replication: NBUK chunk-wise ncfw AllGathers; layer 0
    triggers every chunk DURING the (now fast) transform loop; layers 1/2
    trigger LAG groups after each chunk's rows complete.
  - aggregation: per (6-window group, bucket) one dma_gather pulls 256B src
    rows into G; a one-hot S matrix routes slots into PSUM via bf16 matmuls.
  - HYBRID S sourcing: ~31%% of windows use a HOST-precomputed S slab in HBM
    (nd-dst scale folded into the one-hot values -> epilogue skips the DVE
    multiply); the rest build S on DVE via broadcast is_equal as before.
    S acquisition slides 2 windows ahead of consumption.
  - gather indices int16, SBUF-resident, replicated x8 for the 8 Q7 cores;
    counts unioned across cores so all 8 share one SPMD program.
"""
import sys
sys.path.insert(0, '/opt/trn_rl_repo')

import math
import numpy as np

import concourse.bass as bass
import concourse.bacc as bacc
import concourse.tile as tile
import concourse.mybir as mybir
from concourse.bass_utils import run_bass_kernel_spmd

f32 = mybir.dt.float32
bf16 = mybir.dt.bfloat16
i16 = mybir.dt.int16

NC = 8
GW = 6        # windows per gather group
HEAD = 2      # groups whose gathers are issued before the agg loop
LAG = 2       # groups of lag before triggering next layer's AG chunks
HBM_FRAC = 0.20   # fraction of windows whose S comes from HBM (host-built)


def _preprocess(src, dst, n_nodes):
    src = np.asarray(src).astype(np.int64)
    dst = np.asarray(dst).astype(np.int64)
    N = n_nodes
    assert N % NC == 0
    shard = N // NC
    NW = (shard + 127) // 128
    padshard = NW * 128

    outdeg = np.bincount(src, minlength=N)
    indeg = np.bincount(dst, minlength=N)
    ns = (1.0 / np.sqrt(np.maximum(outdeg, 1))).astype(np.float32)
    nd = (1.0 / np.sqrt(np.maximum(indeg, 1))).astype(np.float32)

    # chunk-major layout: bucket b of the gather table = AllGather output of
    # per-core row chunk b.  Small lead chunk so the first AllGather triggers
    # and drains early; small tail chunk so the last AllGather never gates the
    # next layer; middle chunks as big as int16 gather indices allow.
    if padshard == 12544:
        chs = np.array([1024, 3968, 3968, 2560, 1024], dtype=np.int64)
    else:
        max_chs = (32768 // NC) // 128 * 128
        lead, tail = 1024, 256
        if padshard > lead + tail + 128:
            mid_total = padshard - lead - tail
            n_mid = max(1, math.ceil(mid_total / max_chs))
            mid = [mid_total // n_mid // 128 * 128] * n_mid
            mid[0] += mid_total - sum(mid)
            chs = np.array([lead] + mid + [tail], dtype=np.int64)
        else:
            chs = np.array([padshard], dtype=np.int64)
    NBUK = len(chs)
    assert (chs > 0).all() and (NC * chs).max() <= 32768 and chs.sum() == padshard
    chstart = np.zeros(NBUK + 1, dtype=np.int64)
    chstart[1:] = np.cumsum(chs)
    buksz = [int(NC * c) for c in chs]
    ecore = dst // shard
    NG = (NW + GW - 1) // GW

    cores = []
    cnt_gb = np.zeros((NC, NG, NBUK), dtype=np.int64)
    cnt_wb = np.zeros((NC, NW, NBUK), dtype=np.int64)
    for c in range(NC):
        m = ecore == c
        sc = src[m] // shard
        sr = src[m] % shard
        ld = dst[m] - c * shard
        w = ld >> 7
        slot = ld & 127
        b = np.searchsorted(chstart, sr, side='right') - 1
        reb = sc * chs[b] + (sr - chstart[b])
        g = w // GW
        order = np.lexsort((reb, w, b, g))
        g, b, w, slot, reb = g[order], b[order], w[order], slot[order], reb[order]
        kgb = g * NBUK + b
        cnt_gb[c] = np.bincount(kgb, minlength=NG * NBUK).reshape(NG, NBUK)
        cnt_wb[c] = np.bincount(w * NBUK + b, minlength=NW * NBUK).reshape(NW, NBUK)
        cores.append((kgb, w, b, slot, reb))

    NCOL = (cnt_gb.max(axis=0) + 127) // 128          # [NG, NBUK]
    NCOL[:, 0] = np.maximum(NCOL[:, 0], 1)

    # per-core start offset of window w's run inside its (g,b) call
    start_wb = np.zeros_like(cnt_wb)
    for g_ in range(NG):
        ws = range(g_ * GW, min((g_ + 1) * GW, NW))
        run = np.zeros((NC, NBUK), dtype=np.int64)
        for w_ in ws:
            start_wb[:, w_, :] = run
            run += cnt_wb[:, w_, :]

    # union column range of window w inside call (g,b), across cores
    fc = np.full((NW, NBUK), np.iinfo(np.int64).max, dtype=np.int64)
    lc = np.full((NW, NBUK), -1, dtype=np.int64)
    for c in range(NC):
        has = cnt_wb[c] > 0
        f = start_wb[c] >> 7
        l = (start_wb[c] + cnt_wb[c] - 1) >> 7
        fc[has] = np.minimum(fc[has], f[has])
        lc[has] = np.maximum(lc[has], l[has])
    nvar_wb = np.where(lc >= 0, lc - fc + 1, 0)       # [NW, NBUK]
    C_w = nvar_wb.sum(axis=1)
    assert (C_w >= 1).all()
    TOTCOL = int(C_w.sum())

    # window-major variant column layout (contiguous per window)
    vc_of = np.zeros((NW, NBUK), dtype=np.int64)
    colbase_w = np.zeros(NW, dtype=np.int64)
    acc = 0
    for w_ in range(NW):
        colbase_w[w_] = acc
        for b_ in range(NBUK):
            vc_of[w_, b_] = acc
            acc += int(nvar_wb[w_, b_])
    assert acc == TOTCOL

    # which windows read S from HBM (host-built, nd folded) vs DVE is_eq
    hbm_w = np.zeros(NW, dtype=bool)
    facc = 0.0
    for w_ in range(NW):
        facc += HBM_FRAC
        if facc >= 1.0:
            hbm_w[w_] = True
            facc -= 1.0
    # hbm S column offsets (into S_d) and dve dstl column offsets
    hcol_of = np.zeros(NW, dtype=np.int64)
    dcol_of = np.zeros(NW, dtype=np.int64)
    dve_idx = np.full(NW, -1, dtype=np.int64)   # compact index of DVE windows
    ha = da = nd_i = 0
    for w_ in range(NW):
        if hbm_w[w_]:
            hcol_of[w_] = ha
            ha += int(C_w[w_])
        else:
            dcol_of[w_] = da
            da += int(C_w[w_])
            dve_idx[w_] = nd_i
            nd_i += 1
    HCOLS, DCOLS, NDVE = ha, da, nd_i

    # calls + per-window (variant, G-column) pairing
    groups = []
    call_off = np.zeros((NG, NBUK), dtype=np.int64)
    call_slab = np.zeros((NG, NBUK), dtype=np.int64)
    idx_off = 0
    for g_ in range(NG):
        ws = list(range(g_ * GW, min((g_ + 1) * GW, NW)))
        calls = []
        slabcol = 0
        for b_ in range(NBUK):
            n_cols = int(NCOL[g_, b_])
            if n_cols == 0:
                calls.append(None)
                continue
            call_off[g_, b_] = idx_off
            call_slab[g_, b_] = slabcol
            # descriptors rounded to 16 (idx packing), not 128: un-gathered
            # tail slots of the last column read stale SBUF, which is safe
            # ONLY because the kernel zero-fills all G pool buffers once at
            # startup (virgin SBUF bits can decode as NaN bf16).
            maxcnt = int(cnt_gb[:, g_, b_].max())
            n_idx = min(n_cols * 128, max(16, ((maxcnt + 15) // 16) * 16))
            calls.append((b_, idx_off, n_idx, slabcol))
            idx_off += n_idx
            slabcol += n_cols
        groups.append((ws, calls, slabcol))
    TOTSLOT = idx_off
    assert TOTSLOT % 16 == 0
    C_gmax = max(g[2] for g in groups)

    # size-aware SWDGE queue map: greedy bin-pack buckets over the 4 queues
    # so no queue carries two big buckets' gather drain per group
    nq = min(4, NBUK)
    qload = [0.0] * nq
    qmap = [0] * NBUK
    for b_ in sorted(range(NBUK), key=lambda b: -int(chs[b])):
        q = min(range(nq), key=lambda i: qload[i])
        qmap[b_] = q
        qload[q] += float(chs[b_])

    gcol_of = []
    for g_, (ws, calls, _) in enumerate(groups):
        for w_ in ws:
            lst = []
            for b_ in range(NBUK):
                if calls[b_] is None:
                    continue
                for col in range(int(fc[w_, b_]), int(lc[w_, b_]) + 1):
                    lst.append(int(call_slab[g_, b_]) + col)
            gcol_of.append(lst)

    per_core = []
    lanes = np.arange(128, dtype=np.int64)
    for c in range(NC):
        kgb, w, b, slot, reb = cores[c]
        nkey = NG * NBUK
        run_start = np.zeros(nkey, dtype=np.int64)
        run_start[1:] = np.cumsum(np.bincount(kgb, minlength=nkey))[:-1]
        pos_in_call = np.arange(len(kgb)) - run_start[kgb]
        g_arr = kgb // NBUK
        gslot = call_off[g_arr, b] + pos_in_call

        idx_flat = np.zeros(TOTSLOT, dtype=np.int16)
        idx_flat[gslot] = reb.astype(np.int16)

        col_in_call = pos_in_call >> 7
        lane = pos_in_call & 127
        vc = vc_of[w, b] + (col_in_call - fc[w, b])
        dstl_flat = np.full(TOTCOL * 128, -1.0, dtype=np.float32)
        dstl_flat[vc * 128 + lane] = slot.astype(np.float32)
        dstl2d = dstl_flat.reshape(TOTCOL, 128).T            # [128, TOTCOL]

        idx2d = np.tile(idx_flat.reshape(TOTSLOT // 16, 16).T, (8, 1)).copy()

        ns_sh = np.zeros(padshard, dtype=np.float32)
        nd_sh = np.zeros(padshard, dtype=np.float32)
        ns_sh[:shard] = ns[c * shard:(c + 1) * shard]
        nd_sh[:shard] = nd[c * shard:(c + 1) * shard]
        # layers 1/2 transform scale: ns (this layer's src norm) x nd
        # (previous layer's dst norm, folded past the relu)
        nscol = (ns_sh * nd_sh).reshape(NW, 128).T.copy()
        ndcol = nd_sh.reshape(NW, 128).T.copy()

        # host-built plain one-hot S slab for HBM windows (nd is applied
        # later via the next transform's per-node scale: relu(x*nd)=nd*relu(x)
        # for nd>0, and b0/b1 are spec-pinned zeros)
        S_np = np.zeros((128, HCOLS, 128), dtype=np.float32)
        dstl_dve = np.zeros((128, max(DCOLS, 1)), dtype=np.float32)
        for w_ in range(NW):
            cb = int(colbase_w[w_]); cw = int(C_w[w_])
            if hbm_w[w_]:
                dl = dstl2d[:, cb:cb + cw]                    # [128, cw]
                S_np[:, hcol_of[w_]:hcol_of[w_] + cw, :] = \
                    (dl[:, :, None] == lanes[None, None, :])
            else:
                dstl_dve[:, dcol_of[w_]:dcol_of[w_] + cw] = dstl2d[:, cb:cb + cw]

        S2d = (S_np.reshape(128, HCOLS * 128) if HCOLS
               else np.zeros((128, 128), dtype=np.float32))
        per_core.append(dict(S=S2d,
                             dstl=dstl_dve, idx=idx2d, nscol=nscol,
                             ndcol=ndcol,
                             ns_sh=ns_sh, nd_sh=nd_sh))

    struct = dict(N=N, shard=shard, NW=NW, padshard=padshard,
                  NBUK=NBUK, buksz=buksz, chs=chs, chstart=chstart, C_w=C_w,
                  TOTCOL=TOTCOL, TOTSLOT=TOTSLOT, colbase_w=colbase_w,
                  groups=groups, gcol_of=gcol_of, C_gmax=C_gmax, qmap=qmap,
                  hbm_w=hbm_w, hcol_of=hcol_of, dcol_of=dcol_of,
                  dve_idx=dve_idx, HCOLS=HCOLS, DCOLS=DCOLS, NDVE=NDVE)
    return struct, per_core


def _build_program(st, f_cls):
    NW, padshard = st['NW'], st['padshard']
    NBUK, buksz = st['NBUK'], st['buksz']
    chs, chstart = st['chs'], st['chstart']
    C_w, TOTSLOT = st['C_w'], st['TOTSLOT']
    colbase_w, groups, gcol_of = st['colbase_w'], st['groups'], st['gcol_of']
    shard, C_gmax = st['shard'], st['C_gmax']
    hbm_w, hcol_of, dcol_of = st['hbm_w'], st['hcol_of'], st['dcol_of']
    dve_idx, HCOLS, DCOLS, NDVE = (st['dve_idx'], st['HCOLS'], st['DCOLS'],
                                   st['NDVE'])
    qmap = st['qmap']
    fcp = 64 * ((f_cls + 63) // 64)

    nc = bacc.Bacc(None, target_bir_lowering=False,
                   num_swdge_queues=min(4, NBUK))

    featT_d = nc.dram_tensor("featT", [128, padshard], bf16, kind="ExternalInput")
    idx_d = nc.dram_tensor("idx16", [128, TOTSLOT // 16], i16, kind="ExternalInput")
    S_d = nc.dram_tensor("Shbm", [128, max(HCOLS, 1) * 128 if HCOLS else 128],
                         bf16, kind="ExternalInput")
    dstl_d = nc.dram_tensor("dstl", [128, max(DCOLS, 1)], bf16, kind="ExternalInput")
    iota_d = nc.dram_tensor("iota", [128, 128], bf16, kind="ExternalInput")
    nscol_d = nc.dram_tensor("nscol", [128, NW], f32, kind="ExternalInput")
    ndcol_d = nc.dram_tensor("ndcol", [128, NW], f32, kind="ExternalInput")
    W0_d = nc.dram_tensor("W0", [128, 128], bf16, kind="ExternalInput")
    W1_d = nc.dram_tensor("W1", [128, 128], bf16, kind="ExternalInput")
    W2_d = nc.dram_tensor("W2p", [128, fcp], bf16, kind="ExternalInput")
    b0_d = nc.dram_tensor("b0c", [128, 1], f32, kind="ExternalInput")
    b1_d = nc.dram_tensor("b1c", [128, 1], f32, kind="ExternalInput")
    b2_d = nc.dram_tensor("b2rep", [128, fcp], f32, kind="ExternalInput")
    out_d = nc.dram_tensor("out", [shard, f_cls], f32, kind="ExternalOutput")

    hp0_own = [nc.dram_tensor(f"hp0_own{k}", [int(chs[k]), 128], bf16)
               for k in range(NBUK)]
    hp1_own = [nc.dram_tensor(f"hp1_own{k}", [int(chs[k]), 128], bf16)
               for k in range(NBUK)]
    hp2_own = [nc.dram_tensor(f"hp2_own{k}", [int(chs[k]), 2 * fcp], bf16)
               for k in range(NBUK)]
    hp0_full = [nc.dram_tensor(f"hp0_full{k}", [buksz[k], 128], bf16,
                               addr_space="Shared") for k in range(NBUK)]
    hp1_full = [nc.dram_tensor(f"hp1_full{k}", [buksz[k], 128], bf16,
                               addr_space="Shared") for k in range(NBUK)]
    hp2_full = [nc.dram_tensor(f"hp2_full{k}", [buksz[k], 2 * fcp], bf16,
                               addr_space="Shared") for k in range(NBUK)]

    rg = [list(range(NC))]
    # window after which per-core chunk k's transform rows are complete
    agw = {(int(chstart[k + 1]) - 1) // 128: k for k in range(NBUK)}

    with tile.TileContext(nc) as tc:
        with (
            tc.tile_pool(name="const", bufs=1) as cpool,
            tc.tile_pool(name="gpool", bufs=3) as gpool,
            tc.tile_pool(name="spool", bufs=8) as spool,
            tc.tile_pool(name="wpool", bufs=3) as wpool,
            tc.tile_pool(name="xpool", bufs=3) as xpool,
            tc.tile_pool(name="ftp", bufs=2) as ftp,
            tc.tile_pool(name="psA", bufs=2, space="PSUM") as psA,
            tc.tile_pool(name="psC", bufs=5, space="PSUM") as psC,
        ):
            sW0 = cpool.tile([128, 128], bf16); nc.sync.dma_start(sW0[:], W0_d[:])
            sW1 = cpool.tile([128, 128], bf16); nc.sync.dma_start(sW1[:], W1_d[:])
            sW2 = cpool.tile([128, fcp], bf16); nc.sync.dma_start(sW2[:], W2_d[:])
            sb0 = cpool.tile([128, 1], f32); nc.sync.dma_start(sb0[:], b0_d[:])
            sb1 = cpool.tile([128, 1], f32); nc.sync.dma_start(sb1[:], b1_d[:])
            sb2 = cpool.tile([128, fcp], f32); nc.sync.dma_start(sb2[:], b2_d[:])
            siota = cpool.tile([128, 128], bf16); nc.sync.dma_start(siota[:], iota_d[:])
            sdstl = cpool.tile([128, max(DCOLS, 1)], bf16)
            nc.sync.dma_start(sdstl[:], dstl_d[:])
            snscol = cpool.tile([128, NW], f32); nc.sync.dma_start(snscol[:], nscol_d[:])
            sndcol = cpool.tile([128, NW], f32); nc.sync.dma_start(sndcol[:], ndcol_d[:])
            sidx = cpool.tile([128, TOTSLOT // 16], i16)
            nc.sync.dma_start(sidx[:], idx_d[:])

            def store_rows(hp_own_l, w, hp_tile, p0):
                # store 128 rows of window w from hp_tile[:, p0:p0+128]
                r0 = w * 128
                while r0 < (w + 1) * 128:
                    k = int(np.searchsorted(chstart, r0, side='right')) - 1
                    r1 = min((w + 1) * 128, int(chstart[k + 1]))
                    q0 = r0 - w * 128
                    nc.sync.dma_start(
                        hp_own_l[k][r0 - int(chstart[k]):r1 - int(chstart[k]), :],
                        hp_tile[q0:q0 + (r1 - r0), p0:p0 + 128])
                    r0 = r1

            def ag_chunk(hp_own_l, hp_full_l, k):
                nc.gpsimd.collective_compute(
                    "AllGather", mybir.AluOpType.bypass, rg,
                    ins=[hp_own_l[k][:, :]], outs=[hp_full_l[k][:, :]])

            def transform_single(w, lhsT_ap, sW, hp_own_l):
                ps2 = psA.tile([128, TB * 128], f32, tag="psA")
                nc.tensor.matmul(ps2[:, 0:128], lhsT_ap, sW, start=True, stop=True)
                hp = xpool.tile([128, 128], bf16, tag="hp")
                nc.scalar.activation(hp[:], ps2[:, 0:128],
                                     mybir.ActivationFunctionType.Copy,
                                     scale=snscol[:, w:w + 1])
                store_rows(hp_own_l, w, hp, 0)

            def transform_split(w, lhsT_ap, sW, fo, hp_own_l):
                # layer-2 table rows are [bf16 x fo | zeros x fo] (256B for the
                # gather); the zero half is pre-filled once at startup
                ps2 = psA.tile([128, TB * 128], f32, tag="psA")
                nc.tensor.matmul(ps2[:, 0:fo], lhsT_ap, sW, start=True, stop=True)
                hp = xpool.tile([128, fo], bf16, tag="hp2")
                nc.scalar.activation(hp[:], ps2[:, 0:fo],
                                     mybir.ActivationFunctionType.Copy,
                                     scale=snscol[:, w:w + 1])
                r0 = w * 128
                while r0 < (w + 1) * 128:
                    k = int(np.searchsorted(chstart, r0, side='right')) - 1
                    r1 = min((w + 1) * 128, int(chstart[k + 1]))
                    q0 = r0 - w * 128
                    nc.sync.dma_start(
                        hp_own_l[k][r0 - int(chstart[k]):r1 - int(chstart[k]), 0:fo],
                        hp[q0:q0 + (r1 - r0), :])
                    r0 = r1

            # one-time zero fill of the layer-2 table's pad halves
            zt = cpool.tile([128, fcp], bf16)
            nc.vector.memset(zt[:], 0)
            for k in range(NBUK):
                for r0 in range(0, int(chs[k]), 128):
                    rr = min(128, int(chs[k]) - r0)
                    nc.sync.dma_start(hp2_own[k][r0:r0 + rr, fcp:2 * fcp],
                                      zt[0:rr, :])

            # ---- layer-0 transform: ns pre-folded into featT, batched 4-wide;
            # every AG chunk triggers mid-loop (gpsimd queue is empty here).
            FTB = 8
            TB = 4
            for blk0 in range(0, NW, FTB):
                nwin = min(FTB, NW - blk0)
                ftb = ftp.tile([128, FTB * 128], bf16, tag="ftb")
                nc.sync.dma_start(ftb[:, 0:nwin * 128],
                                  featT_d[:, blk0 * 128:(blk0 + nwin) * 128])
                for sb in range(0, nwin, TB):
                    nb = min(TB, nwin - sb)
                    ps2 = psA.tile([128, TB * 128], f32, tag="psA")
                    for i in range(nb):
                        o = (sb + i) * 128
                        nc.tensor.matmul(ps2[:, i * 128:(i + 1) * 128],
                                         ftb[:, o:o + 128], sW0[:],
                                         start=True, stop=True)
                    hp4 = xpool.tile([128, TB * 128], bf16, tag="hp4")
                    nc.scalar.activation(hp4[:, 0:nb * 128], ps2[:, 0:nb * 128],
                                         mybir.ActivationFunctionType.Copy)
                    for i in range(nb):
                        w = blk0 + sb + i
                        store_rows(hp0_own, w, hp4, i * 128)
                        if w in agw:
                            ag_chunk(hp0_own, hp0_full, agw[w])

            def acquire_S(w):
                cw = int(C_w[w])
                S = spool.tile([128, cw * 128], bf16, tag="S")
                if hbm_w[w]:
                    hb = int(hcol_of[w])
                    nc.sync.dma_start(
                        S[:], S_d[:, hb * 128:(hb + cw) * 128])
                else:
                    db = int(dcol_of[w])
                    in0 = sdstl[:, db:db + cw].unsqueeze(2).broadcast_to([128, cw, 128])
                    in1 = siota[:, :].unsqueeze(1).broadcast_to([128, cw, 128])
                    nc.vector.tensor_tensor(
                        S[:, :].rearrange("p (c x) -> p c x", x=128),
                        in0, in1, mybir.AluOpType.is_equal)
                return S

            def agg_layer(hp_full_l, elem, layer, nxt=None):
                trig = {}
                if nxt is not None:
                    for w_, k_ in agw.items():
                        trig.setdefault(w_ // GW + LAG, []).append(k_)
                done = set()

                def issue_call(G, call, gi):
                    b_, off, n_idx, slabcol = call
                    # size-aware fixed bucket->queue map: keeps each queue's
                    # per-group drain under the group compute period
                    nc.gpsimd.dma_gather(
                        out_ap=G[:, slabcol:slabcol + (n_idx + 127) // 128, :],
                        in_ap=hp_full_l[b_][0:buksz[b_], :],
                        idxs_ap=sidx[:16, off // 16:(off + n_idx) // 16],
                        num_idxs=n_idx,
                        num_idxs_reg=n_idx,
                        elem_size=128,
                        single_packet=False,
                        queue_num=qmap[b_],
                    )

                def issue_group_gathers(gi, G):
                    for call in groups[gi][1]:
                        if call is not None:
                            issue_call(G, call, gi)

                # zero-fill every G pool buffer once so un-gathered tail
                # slots never expose NaN-decoding virgin SBUF to the PE
                if layer == 0:
                    for z in range(3):
                        Gz = gpool.tile([128, C_gmax, 128], bf16, tag="G",
                                        name="Gz")
                        nc.vector.memset(Gz[:], 0)
                # head groups: issue gathers bucket-major so early buckets'
                # gathers cover AllGather chunks still in flight
                Gtiles = {}
                for gi in range(min(HEAD, len(groups))):
                    Gtiles[gi] = gpool.tile([128, groups[gi][2], 128], bf16,
                                            tag="G", name="Gh")
                for b_ in range(NBUK):
                    for gi in list(Gtiles):
                        call = groups[gi][1][b_]
                        if call is not None:
                            issue_call(Gtiles[gi], call, gi)

                # sliding S prefetch: S[w] acquired 6 windows ahead
                SLOOK = 6
                Stiles = {}
                allw = list(range(NW))
                for j in range(min(SLOOK + 1, NW)):
                    Stiles[allw[j]] = acquire_S(allw[j])

                for gi, (ws, calls, C_g) in enumerate(groups):
                    if nxt is not None:
                        for k_ in trig.get(gi, []):
                            ag_chunk(nxt[0], nxt[1], k_)
                            done.add(k_)
                    if gi + 1 < len(groups) and (gi + 1) not in Gtiles:
                        Gtiles[gi + 1] = gpool.tile(
                            [128, groups[gi + 1][2], 128], bf16, tag="G",
                            name="Gn")
                        issue_group_gathers(gi + 1, Gtiles[gi + 1])
                    G = Gtiles.pop(gi)
                    for w in ws:
                        if w + SLOOK + 1 < NW:
                            wn = w + SLOOK + 1
                            Stiles[wn] = acquire_S(wn)
                        S = Stiles.pop(w)
                        cw = int(C_w[w])
                        if layer < 2:
                            ps = psC.tile([128, 128], f32, tag="psC")
                        else:
                            ps = psC.tile([128, elem], f32, tag="psC")
                        for k, gc in enumerate(gcol_of[w]):
                            first = k == 0
                            last = k == len(gcol_of[w]) - 1
                            Sk = S[:, k * 128:(k + 1) * 128]
                            if layer < 2:
                                nc.tensor.matmul(ps[:], G[:, gc, :], Sk,
                                                 start=first, stop=last)
                            else:
                                nc.tensor.matmul(ps[:], Sk, G[:, gc, 0:elem],
                                                 start=first, stop=last)
                        if layer < 2:
                            # b0/b1 are spec-pinned zeros, so relu(agg*nd+b)
                            # = nd*relu(agg) and nd folds into the next
                            # transform's per-node scale (snscol = ns*nd)
                            bias = sb0 if layer == 0 else sb1
                            hsT = wpool.tile([128, 128], bf16, tag="hsT")
                            nc.scalar.activation(
                                hsT[:], ps[:],
                                mybir.ActivationFunctionType.Relu,
                                bias=bias[:])
                            if layer == 0:
                                transform_single(w, hsT[:], sW1[:], hp1_own)
                            else:
                                transform_split(w, hsT[:], sW2[:], fcp, hp2_own)
                        else:
                            rows = min(128, shard - w * 128)
                            t = xpool.tile([128, elem], f32, tag="tout")
                            nc.scalar.activation(
                                t[:], ps[:],
                                mybir.ActivationFunctionType.Copy,
                                scale=sndcol[:, w:w + 1])
                            o = xpool.tile([128, elem], f32, tag="oout")
                            nc.vector.tensor_tensor(
                                o[:], t[:], sb2[:, 0:elem],
                                mybir.AluOpType.add)
                            nc.sync.dma_start(out_d[w * 128:w * 128 + rows, :],
                                              o[:rows, 0:f_cls])
                if nxt is not None:
                    for k_ in range(NBUK):
                        if k_ not in done:
                            ag_chunk(nxt[0], nxt[1], k_)

            agg_layer(hp0_full, 128, 0, nxt=(hp1_own, hp1_full))
            agg_layer(hp1_full, 128, 1, nxt=(hp2_own, hp2_full))
            agg_layer(hp2_full, fcp, 2)

    nc.compile()
    return nc


_cache = {}


def kernel(feat, src, dst, W0, b0, W1, b1, W2, b2):
    import ml_dtypes
    feat = np.ascontiguousarray(feat, dtype=np.float32)
    N = feat.shape[0]
    f_cls = np.asarray(W2).shape[1]
    fcp = 64 * ((f_cls + 63) // 64)

    key = (N, hash(np.asarray(src).tobytes()), hash(np.asarray(dst).tobytes()))
    if key in _cache:
        st, per_core, nc_prog = _cache[key]
    else:
        st, per_core = _preprocess(src, dst, N)
        nc_prog = _build_program(st, f_cls)
        _cache[key] = (st, per_core, nc_prog)

    shard, padshard, NW = st['shard'], st['padshard'], st['NW']
    iota = np.tile(np.arange(128, dtype=np.float32), (128, 1))
    W2p = np.zeros((128, fcp), dtype=np.float32)
    W2p[:, :f_cls] = np.asarray(W2, dtype=np.float32)
    b2rep = np.zeros((128, fcp), dtype=np.float32)
    b2rep[:, :f_cls] = np.asarray(b2, dtype=np.float32)[None, :]
    bfv = lambda a: np.ascontiguousarray(a).astype(ml_dtypes.bfloat16)

    in_maps = []
    for c in range(NC):
        pc = per_core[c]
        # layer-0 ns folded into featT host-side
        featT = np.zeros((128, padshard), dtype=np.float32)
        featT[:, :shard] = feat[c * shard:(c + 1) * shard, :].T
        featT *= pc['ns_sh'][None, :]
        in_maps.append({
            "featT": bfv(featT),
            "idx16": pc['idx'],
            "Shbm": bfv(pc['S']),
            "dstl": bfv(pc['dstl']),
            "iota": bfv(iota),
            "nscol": pc['nscol'],
            "ndcol": pc['ndcol'],
            "W0": bfv(np.asarray(W0, dtype=np.float32)),
            "W1": bfv(np.asarray(W1, dtype=np.float32)),
            "W2p": bfv(W2p),
            "b0c": np.asarray(b0, dtype=np.float32).reshape(128, 1),
            "b1c": np.asarray(b1, dtype=np.float32).reshape(128, 1),
            "b2rep": b2rep,
        })

    import os
    trace = os.environ.get("GCN_TRACE") == "1"
    res = run_bass_kernel_spmd(nc_prog, in_maps, core_ids=list(range(NC)),
                               trace=trace)
    global last_results
    last_results = res
    out = np.concatenate([res.results[c]["out"] for c in range(NC)], axis=0)
    return np.ascontiguousarray(out, dtype=np.float32)


last_results = None
